# revision 1
# baseline (speedup 1.0000x reference)
"""Trainium2 Bass kernel: windowed-LSTM local attention + linear head (LBNER).

Strategy
--------
Data-parallel over batch: B=8 sequences -> 8 NeuronCores, one sequence each.
Per core everything is laid out feature-on-partitions, L=512 on the free dim:

  xT            [768, 512]      (6 SBUF tiles of [128, 512])
  gates/P       [3072, 512]     (24 tiles of [128, 512])
  h, c          [768, 512]      (6 tiles each)

For each window size w in (3,5,7):
  P = Wih @ xT + (b_ih + b_hh)  computed ONCE (shared by all w steps; step t
  just reads P shifted by (t - w//2) columns).  Step t updates only the column
  range [s, e) that is "valid" for that offset, so out-of-range window slots
  never touch state -- this reproduces the reference's mask semantics with no
  mask tensors at all.  Step 0 has h=0 so its hidden matmul is skipped.

Recurrence per step (t >= 1):  gates_psum = WhhT.T @ h  (24 [128,512] psum
tiles, 6 K-chunks each, bf16 x bf16 -> fp32 PSUM), then per d-chunk:
  pre_g = psum + P_shift (DVE)  ->  sigmoid/tanh (ACT)  ->
  c = f*c + i*g (DVE, fp32)     ->  h = o * tanh(c) (DVE, bf16)

After the 3 windows: attn logits via elementwise mul + ones-matmul column
reduction, 3-way softmax on [1,512] rows, attention weights broadcast across
partitions with a K=1 outer-product matmul, and the residual is folded into
the head matmul: logits = lin_w @ xT + lin_w @ (sum_k attn_k * locals_k) + b.

Weights are converted to bf16 on the host; matmul accumulation is fp32 in
PSUM; the cell state c stays fp32; attention/head matmuls run plain fp32.
"""

import math
import numpy as np
import ml_dtypes

import concourse.bacc as bacc
import concourse.bass as bass
import concourse.tile as tile
from concourse import mybir
from concourse import bass_utils

B, L, D = 8, 512, 768
NL = 9
WINDOWS = (3, 5, 7)
NW = len(WINDOWS)
G4 = 4 * D          # 3072
P = 128
ND = D // P         # 6 d-chunks
NM = G4 // P        # 24 gate-chunks
N_CORES = 8

F32 = mybir.dt.float32
F32R = mybir.dt.float32r
BF16 = mybir.dt.bfloat16
AF = mybir.ActivationFunctionType


def _emit(tc, io):
    nc = tc.nc
    from contextlib import ExitStack

    with ExitStack() as ctx:
        const = ctx.enter_context(tc.tile_pool(name="const", bufs=1))
        wpool = ctx.enter_context(tc.tile_pool(name="wpool", bufs=1))
        ppool = ctx.enter_context(tc.tile_pool(name="ppool", bufs=1))
        state = ctx.enter_context(tc.tile_pool(name="state", bufs=1))
        post = ctx.enter_context(tc.tile_pool(name="post", bufs=8))
        tmp = ctx.enter_context(tc.tile_pool(name="tmp", bufs=6))
        attn = ctx.enter_context(tc.tile_pool(name="attn", bufs=7))
        logp = ctx.enter_context(tc.tile_pool(name="logp", bufs=1))
        psum = ctx.enter_context(tc.tile_pool(name="psum", bufs=8, space="PSUM"))

        # ---- constants / inputs resident in SBUF ----
        xf = []   # x.T fp32, for attention dot + residual head matmul
        xb = []   # x.T bf16, rhs of the input projections
        for dc in range(ND):
            t_f = const.tile([P, L], F32, tag=f"xf{dc}")
            nc.sync.dma_start(t_f, io["xf"].ap()[dc * P:(dc + 1) * P, :])
            xf.append(t_f)
            t_b = const.tile([P, L], BF16, tag=f"xb{dc}")
            nc.sync.dma_start(t_b, io["xb"].ap()[dc * P:(dc + 1) * P, :])
            xb.append(t_b)

        # combined LSTM bias, laid out [128, NW, NM]: partition p, window k,
        # gate-chunk m  <-  bias[k, m*128 + p]
        bias_sb = const.tile([P, NW, NM], F32, tag="bias")
        nc.sync.dma_start(
            bias_sb, io["bias"].ap().rearrange("k (m p) -> p k m", p=P)
        )

        lw = []
        for dc in range(ND):
            t = const.tile([P, NL], F32, tag=f"lw{dc}")
            nc.sync.dma_start(t, io["lwt"].ap()[dc * P:(dc + 1) * P, :])
            lw.append(t)
        lb_sb = const.tile([NL, 1], F32, tag="lb")
        nc.sync.dma_start(lb_sb, io["lb"].ap().rearrange("(c o) -> c o", o=1))

        ident_sb = const.tile([P, P], BF16, tag="ident")
        nc.sync.dma_start(ident_sb, io["ident"].ap())

        ones_col = const.tile([P, 1], F32, tag="ones_col")
        nc.vector.memset(ones_col, 1.0)
        ones_row = const.tile([1, P], F32, tag="ones_row")
        nc.vector.memset(ones_row, 1.0)

        locals_k = []   # per window: list of 6 bf16 [128, 512] tiles (final h)
        a_sb = []       # per-window attention logit rows [1, 512]
        inv_sqrt_d = 1.0 / math.sqrt(D)

        for k, w in enumerate(WINDOWS):
            hw_ = w // 2

            # ---- weights for this window (2 rotating 9.4MB slots) ----
            wih = []
            for kc in range(ND):
                t = wpool.tile([P, G4], BF16, tag=f"A{kc}")
                nc.sync.dma_start(t, io["wih"].ap()[k, kc * P:(kc + 1) * P, :])
                wih.append(t)
            whh = []
            for kc in range(ND):
                t = wpool.tile([P, G4], BF16, tag=f"B{kc}")
                nc.sync.dma_start(t, io["whh"].ap()[k, kc * P:(kc + 1) * P, :])
                whh.append(t)

            # ---- input projection: P_m = bias_m + sum_kc Wih[kc,m].T @ xT ----
            Pt = []
            for m in range(NM):
                ps = psum.tile([P, L], F32, tag="g")
                for kc in range(ND):
                    nc.tensor.matmul(
                        ps,
                        lhsT=wih[kc][:, m * P:(m + 1) * P],
                        rhs=xb[kc][:],
                        start=(kc == 0),
                        stop=(kc == ND - 1),
                    )
                pt = ppool.tile([P, L], BF16, tag=f"P{m}")
                nc.vector.tensor_scalar_add(pt, ps, bias_sb[:, k, m:m + 1])
                Pt.append(pt)

            # ---- state init ----
            c = []
            h = []
            for dc in range(ND):
                ct = state.tile([P, L], F32, tag=f"c{dc}")
                nc.gpsimd.memset(ct, 0.0)
                c.append(ct)
                ht = state.tile([P, L], BF16, tag=f"loc{k}_{dc}")
                nc.gpsimd.memset(ht, 0.0)
                h.append(ht)

            # ---- recurrence over window positions ----
            for t in range(w):
                off = t - hw_
                s = max(0, -off)
                e = min(L, L - off)
                n = e - s

                if t == 0:
                    # h == 0: gates come straight from P (bias included)
                    for dc in range(ND):
                        i_t = post.tile([P, L], BF16, tag="post")
                        nc.scalar.activation(
                            i_t[:, :n], Pt[0 + dc][:, s + off:e + off], AF.Sigmoid
                        )
                        g_t = post.tile([P, L], BF16, tag="post")
                        nc.scalar.activation(
                            g_t[:, :n], Pt[12 + dc][:, s + off:e + off], AF.Tanh
                        )
                        o_t = post.tile([P, L], BF16, tag="post")
                        nc.scalar.activation(
                            o_t[:, :n], Pt[18 + dc][:, s + off:e + off], AF.Sigmoid
                        )
                        nc.vector.tensor_mul(c[dc][:, s:e], i_t[:, :n], g_t[:, :n])
                        tc_t = post.tile([P, L], BF16, tag="post")
                        nc.scalar.activation(tc_t[:, :n], c[dc][:, s:e], AF.Tanh)
                        nc.vector.tensor_mul(h[dc][:, s:e], o_t[:, :n], tc_t[:, :n])
                    continue

                for dc in range(ND):
                    # 4 gate psum tiles for this d-chunk: i, f, g, o.
                    # P_shift (incl. bias) is folded into the accumulation
                    # with an identity matmul, so ACT reads gates from PSUM.
                    gp = []
                    for base in (0, 6, 12, 18):
                        m = base + dc
                        ps = psum.tile([P, L], F32, tag="g")
                        nc.tensor.matmul(
                            ps[:, s:e],
                            lhsT=ident_sb[:],
                            rhs=Pt[m][:, s + off:e + off],
                            start=True,
                            stop=False,
                        )
                        for kc in range(ND):
                            nc.tensor.matmul(
                                ps[:, s:e],
                                lhsT=whh[kc][:, m * P:(m + 1) * P],
                                rhs=h[kc][:, s:e],
                                start=False,
                                stop=(kc == ND - 1),
                            )
                        gp.append(ps)

                    acts = []
                    for gi, fn in enumerate(
                        (AF.Sigmoid, AF.Sigmoid, AF.Tanh, AF.Sigmoid)
                    ):
                        a = post.tile([P, L], BF16, tag="post")
                        nc.scalar.activation(a[:, :n], gp[gi][:, s:e], fn)
                        acts.append(a)
                    i_t, f_t, g_t, o_t = acts

                    t1 = tmp.tile([P, L], F32, tag="tmp")
                    nc.vector.tensor_mul(t1[:, :n], i_t[:, :n], g_t[:, :n])
                    t2 = tmp.tile([P, L], F32, tag="tmp")
                    nc.vector.tensor_mul(t2[:, :n], f_t[:, :n], c[dc][:, s:e])
                    nc.vector.tensor_add(c[dc][:, s:e], t1[:, :n], t2[:, :n])
                    tc_t = post.tile([P, L], BF16, tag="post")
                    nc.scalar.activation(tc_t[:, :n], c[dc][:, s:e], AF.Tanh)
                    nc.vector.tensor_mul(h[dc][:, s:e], o_t[:, :n], tc_t[:, :n])

            locals_k.append(h)

            # attention dot for this window, overlapped with the next window
            psd = psum.tile([1, L], F32, tag="g")
            for dc in range(ND):
                td = tmp.tile([P, L], F32, tag="tmp")
                nc.vector.tensor_mul(td, xf[dc][:], h[dc][:])
                nc.tensor.matmul(
                    psd,
                    lhsT=ones_col[:],
                    rhs=td[:],
                    start=(dc == 0),
                    stop=(dc == ND - 1),
                )
            ak = attn.tile([1, L], F32, tag=f"ak{k}", bufs=1)
            nc.scalar.activation(ak, psd, AF.Copy, scale=inv_sqrt_d)
            a_sb.append(ak)

        # ---- attention over the 3 window outputs ----
        mx1 = attn.tile([1, L], F32, tag="sm")
        nc.vector.tensor_max(mx1, a_sb[0][:], a_sb[1][:])
        mx2 = attn.tile([1, L], F32, tag="sm")
        nc.vector.tensor_max(mx2, mx1[:], a_sb[2][:])
        d_sb = []
        for k in range(NW):
            d_k = attn.tile([1, L], F32, tag="sm")
            nc.vector.tensor_sub(d_k, a_sb[k][:], mx2[:])
            d_sb.append(d_k)
        e_sb = []
        for k in range(NW):
            ek = attn.tile([1, L], F32, tag="sm")
            nc.scalar.activation(ek, d_sb[k][:], AF.Exp)
            e_sb.append(ek)
        s1 = attn.tile([1, L], F32, tag="sm")
        nc.vector.tensor_add(s1, e_sb[0][:], e_sb[1][:])
        s2 = attn.tile([1, L], F32, tag="sm")
        nc.vector.tensor_add(s2, s1[:], e_sb[2][:])
        r = attn.tile([1, L], F32, tag="sm")
        nc.vector.reciprocal(r, s2[:])

        wb = []   # attention weights broadcast to [128, 512] (PSUM)
        for k in range(NW):
            wk = attn.tile([1, L], F32, tag="sm")
            nc.vector.tensor_mul(wk, e_sb[k][:], r[:])
            pb = psum.tile([P, L], F32, tag="g")
            nc.tensor.matmul(
                pb,
                lhsT=ones_row[:],
                rhs=wk[:],
                start=True,
                stop=True,
            )
            wb.append(pb)

        # ---- head: logits = lin_w @ (x + sum_k attn_k * locals_k) + b ----
        ps_log = psum.tile([NL, L], F32, tag="g")
        for dc in range(ND):
            nc.tensor.matmul(
                ps_log,
                lhsT=lw[dc][:],
                rhs=xf[dc][:],
                start=(dc == 0),
                stop=False,
            )
        for dc in range(ND):
            lf = tmp.tile([P, L], F32, tag="tmp")
            nc.vector.tensor_mul(lf, wb[0][:], locals_k[0][dc][:])
            t3 = tmp.tile([P, L], F32, tag="tmp")
            nc.vector.tensor_mul(t3, wb[1][:], locals_k[1][dc][:])
            lf2 = tmp.tile([P, L], F32, tag="tmp")
            nc.vector.tensor_add(lf2, lf[:], t3[:])
            t4 = tmp.tile([P, L], F32, tag="tmp")
            nc.vector.tensor_mul(t4, wb[2][:], locals_k[2][dc][:])
            lf3 = tmp.tile([P, L], F32, tag="tmp")
            nc.vector.tensor_add(lf3, lf2[:], t4[:])
            nc.tensor.matmul(
                ps_log,
                lhsT=lw[dc][:],
                rhs=lf3[:],
                start=False,
                stop=(dc == ND - 1),
            )
        logits = logp.tile([NL, L], F32, tag="logits")
        nc.scalar.activation(logits, ps_log, AF.Identity, bias=lb_sb[:, 0:1])
        # store transposed: out[l, c] = logits[c, l]
        nc.sync.dma_start(io["out"].ap().rearrange("l c -> c l"), logits[:])


_NC_CACHE = {}


def _get_nc():
    if "nc" not in _NC_CACHE:
        nc = bacc.Bacc("TRN2", target_bir_lowering=False, debug=False)
        io = {
            "xf": nc.dram_tensor("xf", [D, L], F32, kind="ExternalInput"),
            "xb": nc.dram_tensor("xb", [D, L], BF16, kind="ExternalInput"),
            "wih": nc.dram_tensor("wih", [NW, D, G4], BF16, kind="ExternalInput"),
            "whh": nc.dram_tensor("whh", [NW, D, G4], BF16, kind="ExternalInput"),
            "bias": nc.dram_tensor("bias", [NW, G4], F32, kind="ExternalInput"),
            "lwt": nc.dram_tensor("lwt", [D, NL], F32, kind="ExternalInput"),
            "lb": nc.dram_tensor("lb", [NL], F32, kind="ExternalInput"),
            "ident": nc.dram_tensor("ident", [P, P], BF16, kind="ExternalInput"),
            "out": nc.dram_tensor("out", [L, NL], F32, kind="ExternalOutput"),
        }
        with tile.TileContext(nc) as tc:
            _emit(tc, io)
        nc.compile()
        _NC_CACHE["nc"] = nc
    return _NC_CACHE["nc"]


def _in_maps(sequence_output, W_ih, W_hh, b_ih, b_hh, lin_w, lin_b):
    x = np.asarray(sequence_output, np.float32)
    WihT = np.ascontiguousarray(
        np.transpose(np.asarray(W_ih, np.float32), (0, 2, 1))
    ).astype(ml_dtypes.bfloat16)
    WhhT = np.ascontiguousarray(
        np.transpose(np.asarray(W_hh, np.float32), (0, 2, 1))
    ).astype(ml_dtypes.bfloat16)
    biasc = np.asarray(b_ih, np.float32) + np.asarray(b_hh, np.float32)
    lwt = np.ascontiguousarray(np.asarray(lin_w, np.float32).T)
    lb = np.asarray(lin_b, np.float32)
    maps = []
    for b in range(B):
        xT = np.ascontiguousarray(x[b].T)
        maps.append({
            "xf": xT,
            "xb": xT.astype(ml_dtypes.bfloat16),
            "wih": WihT,
            "whh": WhhT,
            "bias": biasc,
            "lwt": lwt,
            "lb": lb,
            "ident": np.eye(P, dtype=np.float32).astype(ml_dtypes.bfloat16),
        })
    return maps


def kernel(sequence_output, W_ih, W_hh, b_ih, b_hh, lin_w, lin_b):
    nc = _get_nc()
    maps = _in_maps(sequence_output, W_ih, W_hh, b_ih, b_hh, lin_w, lin_b)
    res = bass_utils.run_bass_kernel_spmd(nc, maps, core_ids=list(range(N_CORES)))
    return np.stack([res.results[b]["out"] for b in range(B)], axis=0)


def run_traced(inputs, **kw):
    """For test.py: run with NTFF tracing, returns BassKernelResults."""
    nc = _get_nc()
    maps = _in_maps(**inputs)
    return bass_utils.run_bass_kernel_spmd(
        nc, maps, core_ids=list(range(N_CORES)), trace=True, **kw
    )



# revision 4
# speedup vs baseline: 1.6974x; 1.6974x over previous
"""Trainium2 Bass kernel v2: windowed-LSTM local attention + linear head.

Key changes vs v1:
- fp8(e4m3) DoubleRow matmuls for both input and hidden projections: 2x PE
  throughput, half the weight DMA (14MB vs 28MB), all 3 windows' weights
  resident in SBUF.
- Weights scaled x16 and h scaled x4 on host to keep fp8 values in the
  normal range; the combined x64 is undone in the ACT gate read (scale=1/64).
  P (input projection) stored fp8 at x16; injected into PSUM via a 4*I fp8
  matmul (4*16 = 64 matches the hidden-product scaling).
- Cell math split across engines: ACT sigmoid/tanh from PSUM, DVE bf16
  elementwise (4x mode), Pool (gpsimd) produces the fp8 h state.
- Windows processed sequentially but next window's input projection is
  emitted as PE filler between recurrence steps.
- Attention/head restructured: logits = lw@x + sum_k softmax_k (*) (lw@h_k),
  computed in [9, L] space - no 128-partition broadcasts, no transposing
  output DMA (host transposes the [9, L] result).
"""

import math
import numpy as np
import ml_dtypes

import concourse.bacc as bacc
import concourse.bass as bass
import concourse.tile as tile
from concourse import mybir
from concourse import bass_utils
from concourse.alu_op_type import AluOpType

B, L, D = 8, 512, 768
NL = 9
WINDOWS = (3, 5, 7)
NW = len(WINDOWS)
G4 = 4 * D          # 3072
P = 128
ND = D // P         # 6 k-chunks of the contraction dim
NM = G4 // P        # 24 gate-chunks
NJ = ND // 2        # 3 DoubleRow k-pairs
N_CORES = 8

SW = 16.0           # host weight scale (fp8 subnormal avoidance)
SH = 4.0            # h state scale
SINJ = SW * SH      # PSUM scale of hidden products = 64

F32 = mybir.dt.float32
BF16 = mybir.dt.bfloat16
F8 = mybir.dt.float8e4
AF = mybir.ActivationFunctionType
DR = mybir.MatmulPerfMode.DoubleRow

NPF8 = mybir.dt.np(F8)
NPBF = mybir.dt.np(BF16)


def _emit(tc, io):
    nc = tc.nc
    from contextlib import ExitStack

    with ExitStack() as ctx:
        const = ctx.enter_context(tc.tile_pool(name="const", bufs=1))
        wpool = ctx.enter_context(tc.tile_pool(name="wpool", bufs=1))
        ppool = ctx.enter_context(tc.tile_pool(name="ppool", bufs=1))
        state = ctx.enter_context(tc.tile_pool(name="state", bufs=1))
        gpool = ctx.enter_context(tc.tile_pool(name="gates", bufs=1))
        tpool = ctx.enter_context(tc.tile_pool(name="tmp", bufs=4))
        apool = ctx.enter_context(tc.tile_pool(name="attn", bufs=1))
        psum = ctx.enter_context(tc.tile_pool(name="psum", bufs=8, space="PSUM"))

        # ---- resident constants ----
        # DMA order matters: the critical path to the first gate activation
        # is xq -> wih0 -> (proj) -> step0, so those go first; bulk tensors
        # that are only needed later are deferred below.
        xq = const.tile([P, ND, L], F8, tag="xq")
        nc.sync.dma_start(xq, io["xq"].ap())
        bias_sb = const.tile([P, NW, NM], F32, tag="bias")
        nc.sync.dma_start(bias_sb, io["bias16"].ap())
        wi0 = wpool.tile([P, ND, G4], F8, tag="wi")
        for j in range(NJ):  # split so early proj matmuls start sooner
            nc.sync.dma_start(wi0[:, 2 * j:2 * j + 2, :],
                              io["wih"].ap()[0, :, 2 * j:2 * j + 2, :])
        ident4 = const.tile([P, P], F8, tag="ident4")
        nc.sync.dma_start(ident4, io["ident4"].ap())
        whh = []
        for k in range(NW):
            t = wpool.tile([P, ND, G4], F8, tag=f"whh{k}")
            whh.append(t)
        nc.sync.dma_start(whh[0], io["whh"].ap()[0])
        xb = const.tile([P, ND, L], BF16, tag="xb")
        nc.sync.dma_start(xb, io["xb"].ap())
        nc.sync.dma_start(whh[1], io["whh"].ap()[1])
        nc.sync.dma_start(whh[2], io["whh"].ap()[2])
        lwt = const.tile([P, ND, NL], BF16, tag="lwt")
        nc.sync.dma_start(lwt, io["lwt"].ap())
        lb_sb = const.tile([NL, 1], F32, tag="lb")
        nc.sync.dma_start(lb_sb, io["lb"].ap())
        ones_col = const.tile([P, 1], BF16, tag="ones_col")
        nc.vector.memset(ones_col, 1.0)
        ones9 = const.tile([1, NL], BF16, tag="ones9")
        nc.vector.memset(ones9, 1.0)

        Ps = [ppool.tile([P, NM, L], F8, tag=f"P{k}", name=f"P{k}")
              for k in range(NW)]
        h_s = []
        for k in range(NW):
            t = state.tile([P, ND, L], F8, tag=f"h{k}")
            h_s.append(t)

        inv_sqrt_d = 1.0 / math.sqrt(D)

        # ---------- emission helpers ----------
        def emit_wi_dma(k):
            wi = wpool.tile([P, ND, G4], F8, tag="wi")
            nc.sync.dma_start(wi, io["wih"].ap()[k])
            return wi

        def emit_proj_chunk(k, wi, m):
            """P_s[k][:, m, :] = fp8( (16*Wih)@x + 16*bias )"""
            ps = psum.tile([P, L], F32, tag="g")
            for j in range(NJ):
                nc.tensor.matmul(
                    ps,
                    lhsT=wi[:, 2 * j:2 * j + 2, m * P:(m + 1) * P],
                    rhs=xq[:, 2 * j:2 * j + 2, :],
                    start=(j == 0),
                    stop=(j == NJ - 1),
                    perf_mode=DR,
                )
            eng = (nc.vector, nc.scalar, nc.vector)[m % 3]
            if eng is nc.scalar:
                nc.scalar.activation(Ps[k][:, m, :], ps, AF.Identity,
                                     bias=bias_sb[:, k, m:m + 1])
            else:
                eng.tensor_scalar_add(Ps[k][:, m, :], ps, bias_sb[:, k, m:m + 1])

        def emit_state_init(k, w):
            hw_ = w // 2
            c = state.tile([P, ND, L], BF16, tag="c")
            nc.vector.memset(c[:, :, 0:hw_], 0.0)
            nc.vector.memset(h_s[k][:, :, 0:hw_], 0.0)
            return c

        def emit_cell(k, c, dc, s, e, n, ia, ga, oa, first):
            """c/h update for one d-chunk (after its gate activations)."""
            if first:
                nc.vector.tensor_mul(c[:, dc, s:e], ia[:, dc, s:e], ga[:, dc, s:e])
            tch = tpool.tile([P, L], BF16, tag="tch")
            nc.scalar.activation(tch[:, :n], c[:, dc, s:e], AF.Tanh)
            # scalar_tensor_tensor is DVE-only (Pool/GPSIMD lacks the opcode)
            nc.vector.scalar_tensor_tensor(
                h_s[k][:, dc, s:e], tch[:, :n], SH, oa[:, dc, s:e],
                op0=AluOpType.mult, op1=AluOpType.mult,
            )

        def emit_step0(k, w, c):
            hw_ = w // 2
            s, e = hw_, L           # off = -hw: cols [hw, L)
            n = e - s
            off = -hw_
            ia = gpool.tile([P, ND, L], BF16, tag="ia")
            ga = gpool.tile([P, ND, L], BF16, tag="ga")
            oa = gpool.tile([P, ND, L], BF16, tag="oa")
            for dc in range(ND):
                nc.scalar.activation(
                    ia[:, dc, s:e], Ps[k][:, 0 + dc, s + off:e + off],
                    AF.Sigmoid, scale=1.0 / SW)
                nc.scalar.activation(
                    ga[:, dc, s:e], Ps[k][:, 12 + dc, s + off:e + off],
                    AF.Tanh, scale=1.0 / SW)
                nc.scalar.activation(
                    oa[:, dc, s:e], Ps[k][:, 18 + dc, s + off:e + off],
                    AF.Sigmoid, scale=1.0 / SW)
                if dc >= 1:
                    emit_cell(k, c, dc - 1, s, e, n, ia, ga, oa, first=True)
            emit_cell(k, c, ND - 1, s, e, n, ia, ga, oa, first=True)

        def emit_step(k, w, t, c):
            hw_ = w // 2
            off = t - hw_
            s = max(0, -off)
            e = min(L, L - off)
            n = e - s
            # PE: 24 psum groups in the order ACT will drain them
            pss = {}
            for dc in range(ND):
                for base in (0, 6, 12, 18):
                    m = base + dc
                    ps = psum.tile([P, L], F32, tag="g")
                    nc.tensor.matmul(
                        ps[:, s:e],
                        lhsT=ident4[:],
                        rhs=Ps[k][:, m, s + off:e + off],
                        start=True,
                        stop=False,
                    )
                    for j in range(NJ):
                        nc.tensor.matmul(
                            ps[:, s:e],
                            lhsT=whh[k][:, 2 * j:2 * j + 2, m * P:(m + 1) * P],
                            rhs=h_s[k][:, 2 * j:2 * j + 2, s:e],
                            start=False,
                            stop=(j == NJ - 1),
                            perf_mode=DR,
                        )
                    pss[m] = ps
            ia = gpool.tile([P, ND, L], BF16, tag="ia")
            fa = gpool.tile([P, ND, L], BF16, tag="fa")
            ga = gpool.tile([P, ND, L], BF16, tag="ga")
            oa = gpool.tile([P, ND, L], BF16, tag="oa")
            inv = 1.0 / SINJ
            for dc in range(ND):
                nc.scalar.activation(ia[:, dc, s:e], pss[0 + dc][:, s:e],
                                     AF.Sigmoid, scale=inv)
                nc.scalar.activation(fa[:, dc, s:e], pss[6 + dc][:, s:e],
                                     AF.Sigmoid, scale=inv)
                nc.scalar.activation(ga[:, dc, s:e], pss[12 + dc][:, s:e],
                                     AF.Tanh, scale=inv)
                nc.scalar.activation(oa[:, dc, s:e], pss[18 + dc][:, s:e],
                                     AF.Sigmoid, scale=inv)
                if dc >= 1:
                    dcp = dc - 1
                    ta = tpool.tile([P, L], BF16, tag="t")
                    nc.vector.tensor_mul(ta[:, :n], ia[:, dcp, s:e], ga[:, dcp, s:e])
                    tb = tpool.tile([P, L], BF16, tag="t")
                    nc.vector.tensor_mul(tb[:, :n], fa[:, dcp, s:e], c[:, dcp, s:e])
                    nc.vector.tensor_add(c[:, dcp, s:e], ta[:, :n], tb[:, :n])
                    emit_cell(k, c, dcp, s, e, n, ia, ga, oa, first=False)
            dcp = ND - 1
            ta = tpool.tile([P, L], BF16, tag="t")
            nc.vector.tensor_mul(ta[:, :n], ia[:, dcp, s:e], ga[:, dcp, s:e])
            tb = tpool.tile([P, L], BF16, tag="t")
            nc.vector.tensor_mul(tb[:, :n], fa[:, dcp, s:e], c[:, dcp, s:e])
            nc.vector.tensor_add(c[:, dcp, s:e], ta[:, :n], tb[:, :n])
            emit_cell(k, c, dcp, s, e, n, ia, ga, oa, first=False)

        e_sb = {}
        y_sb = {}

        def emit_tail(k):
            """attention dot + head projection of this window's locals."""
            a_ps = psum.tile([1, L], F32, tag="g")
            y_ps = psum.tile([NL, L], F32, tag="g")
            for dc in range(ND):
                hb = tpool.tile([P, L], BF16, tag="hb", bufs=4)
                nc.gpsimd.tensor_scalar_mul(hb, h_s[k][:, dc, :], 1.0 / SH)
                td = tpool.tile([P, L], BF16, tag="td", bufs=4)
                nc.vector.tensor_mul(td, xb[:, dc, :], hb[:])
                nc.tensor.matmul(a_ps, lhsT=ones_col[:], rhs=td[:],
                                 start=(dc == 0), stop=(dc == ND - 1))
                nc.tensor.matmul(y_ps, lhsT=lwt[:, dc, :], rhs=hb[:],
                                 start=(dc == 0), stop=(dc == ND - 1))
            ek = apool.tile([1, L], F32, tag=f"e{k}", name=f"e{k}")
            nc.scalar.activation(ek, a_ps, AF.Exp, scale=inv_sqrt_d)
            e_sb[k] = ek
            yk = apool.tile([NL, L], F32, tag=f"y{k}", name=f"y{k}")
            nc.vector.tensor_copy(yk, y_ps)
            y_sb[k] = yk

        pre = {}

        def emit_s1():
            s1 = apool.tile([1, L], F32, tag="sm", bufs=3)
            nc.vector.tensor_add(s1, e_sb[0][:], e_sb[1][:])
            pre["s1"] = s1

        def emit_hp():
            hp = psum.tile([NL, L], F32, tag="g")
            for dc in range(ND):
                nc.tensor.matmul(hp, lhsT=lwt[:, dc, :], rhs=xb[:, dc, :],
                                 start=(dc == 0), stop=(dc == ND - 1))
            pre["hp"] = hp

        def emit_final():
            s2 = apool.tile([1, L], F32, tag="sm", bufs=3)
            nc.vector.tensor_add(s2, pre["s1"][:], e_sb[2][:])
            r = apool.tile([1, L], F32, tag="sm", bufs=3)
            nc.vector.reciprocal(r, s2[:])
            wbs = []
            for k in range(NW):
                wn = apool.tile([1, L], BF16, tag="wn", bufs=3)
                nc.vector.tensor_mul(wn, e_sb[k][:], r[:])
                wb = psum.tile([NL, L], F32, tag="g")
                nc.tensor.matmul(wb, lhsT=ones9[:], rhs=wn[:],
                                 start=True, stop=True)
                wbs.append(wb)
            hp = pre["hp"]
            m0 = apool.tile([NL, L], F32, tag="fin", bufs=4)
            nc.vector.tensor_mul(m0, wbs[0][:], y_sb[0][:])
            m1 = apool.tile([NL, L], F32, tag="fin", bufs=4)
            nc.vector.tensor_mul(m1, wbs[1][:], y_sb[1][:])
            a01 = apool.tile([NL, L], F32, tag="fin", bufs=4)
            nc.vector.tensor_add(a01, m0[:], m1[:])
            m2 = apool.tile([NL, L], F32, tag="fin", bufs=4)
            nc.vector.tensor_mul(m2, wbs[2][:], y_sb[2][:])
            a012 = apool.tile([NL, L], F32, tag="fin", bufs=4)
            nc.vector.tensor_add(a012, a01[:], m2[:])
            af = apool.tile([NL, L], F32, tag="fin", bufs=4)
            nc.vector.tensor_add(af, a012[:], hp[:])
            logits = apool.tile([NL, L], F32, tag="fin", bufs=4)
            nc.scalar.activation(logits, af[:], AF.Identity, bias=lb_sb[:, 0:1])
            nc.sync.dma_start(io["out"].ap(), logits[:])

        # ---------- schedule ----------
        fillers = []

        def drain_fillers(nmax):
            nonlocal fillers
            take, fillers = fillers[:nmax], fillers[nmax:]
            for f in take:
                f()

        # w0 projection inline
        for m in range(NM):
            emit_proj_chunk(0, wi0, m)

        PER_GAP = 6
        for k, w in enumerate(WINDOWS):
            if k + 1 < NW:
                # queue next window's projection as PE filler work
                win = emit_wi_dma(k + 1)
                fillers += [
                    (lambda kk=k + 1, wii=win, mm=m: emit_proj_chunk(kk, wii, mm))
                    for m in range(NM)
                ]
            else:
                fillers.append(emit_hp)
            c = emit_state_init(k, w)
            emit_step0(k, w, c)
            for t in range(1, w):
                # in window 0, give the wi1 DMA a step of headroom first
                if k > 0 or t >= 2:
                    drain_fillers(PER_GAP)
                emit_step(k, w, t, c)
            drain_fillers(len(fillers))
            if k == 1:
                emit_tail(k)
                emit_s1()
            else:
                emit_tail(k)
        emit_final()


_NC_CACHE = {}


def _build_nc():
    nc = bacc.Bacc("TRN2", target_bir_lowering=False, debug=False)
    io = {
        "xq": nc.dram_tensor("xq", [P, ND, L], F8, kind="ExternalInput"),
        "xb": nc.dram_tensor("xb", [P, ND, L], BF16, kind="ExternalInput"),
        "wih": nc.dram_tensor("wih", [NW, P, ND, G4], F8, kind="ExternalInput"),
        "whh": nc.dram_tensor("whh", [NW, P, ND, G4], F8, kind="ExternalInput"),
        "bias16": nc.dram_tensor("bias16", [P, NW, NM], F32, kind="ExternalInput"),
        "lwt": nc.dram_tensor("lwt", [P, ND, NL], BF16, kind="ExternalInput"),
        "lb": nc.dram_tensor("lb", [NL, 1], F32, kind="ExternalInput"),
        "ident4": nc.dram_tensor("ident4", [P, P], F8, kind="ExternalInput"),
        "out": nc.dram_tensor("out", [NL, L], F32, kind="ExternalOutput"),
    }
    with tile.TileContext(nc) as tc:
        _emit(tc, io)
    nc.compile()
    return nc


def _get_nc():
    if "nc" not in _NC_CACHE:
        _NC_CACHE["nc"] = _build_nc()
    return _NC_CACHE["nc"]


def _in_maps(sequence_output, W_ih, W_hh, b_ih, b_hh, lin_w, lin_b):
    x = np.asarray(sequence_output, np.float32)

    def kblocked(a):   # [C, F] -> [128, C//128, F]
        C, F = a.shape
        return np.ascontiguousarray(
            a.reshape(C // P, P, F).transpose(1, 0, 2))

    wih_l = np.stack([
        kblocked(np.asarray(W_ih[k], np.float32).T * SW) for k in range(NW)
    ]).astype(NPF8)
    whh_l = np.stack([
        kblocked(np.asarray(W_hh[k], np.float32).T * SW) for k in range(NW)
    ]).astype(NPF8)
    biasc = (np.asarray(b_ih, np.float32) + np.asarray(b_hh, np.float32)) * SW
    bias16 = np.ascontiguousarray(
        biasc.reshape(NW, NM, P).transpose(2, 0, 1))     # [128, 3, 24]
    lwt = kblocked(np.asarray(lin_w, np.float32).T).astype(NPBF)  # [128, 6, 9]
    lb = np.asarray(lin_b, np.float32).reshape(NL, 1)
    ident4 = (SH * np.eye(P, dtype=np.float32)).astype(NPF8)

    maps = []
    for b in range(B):
        xT = np.ascontiguousarray(x[b].T)                # [768, 512]
        xkb = kblocked(xT)                               # [128, 6, 512]
        maps.append({
            "xq": xkb.astype(NPF8),
            "xb": xkb.astype(NPBF),
            "wih": wih_l,
            "whh": whh_l,
            "bias16": bias16,
            "lwt": lwt,
            "lb": lb,
            "ident4": ident4,
        })
    return maps


def kernel(sequence_output, W_ih, W_hh, b_ih, b_hh, lin_w, lin_b):
    nc = _get_nc()
    maps = _in_maps(sequence_output, W_ih, W_hh, b_ih, b_hh, lin_w, lin_b)
    res = bass_utils.run_bass_kernel_spmd(nc, maps, core_ids=list(range(N_CORES)))
    return np.stack(
        [np.ascontiguousarray(res.results[b]["out"].T) for b in range(B)], axis=0
    )


def run_traced(inputs, **kw):
    nc = _get_nc()
    maps = _in_maps(**inputs)
    return bass_utils.run_bass_kernel_spmd(
        nc, maps, core_ids=list(range(N_CORES)), trace=True, **kw
    )
